# revision 12
# baseline (speedup 1.0000x reference)
"""Paged KV-cache gather + dequant kernel for 8 Trainium2 NeuronCores.

Problem: out[0] = zeros; out[1+i] = kv_cache[block_tables.flat[i]] * scale
(k_scale for the K half, v_scale for the V half), zeroed where the table
entry is <= 0.  Shapes: kv_cache [4096, 2, 8, 16, 128] fp16,
block_tables [32, 128] int, out [4097, 2, 8, 16, 128] fp16.

Sharding: batch across the 8 cores (4 sequences = 512 entries per core);
kv_cache replicated.  Per core the kernel views kv_cache as 8192 rows of
16384 fp16 (one row = one K or V half-block, 32 KB) and:
  1. loads block_tables, builds int16 row indices (2*bt for K, 2*bt+1 for V)
     wrapped in 16 partitions and replicated across the 8 GPSIMD cores,
  2. dma_gather's 128 rows per call into [128, 16384] SBUF tiles,
  3. multiplies by a per-partition scalar (valid * k/v_scale) on DVE,
  4. stores rows to the output shard with a strided HWDGE DMA.
Invalid entries gather row 0/1 and are zeroed by the scale; output block 0
is never written (ExternalOutput buffers are zero-initialized).
"""

import sys

if "/opt/trn_rl_repo" not in sys.path:
    sys.path.insert(0, "/opt/trn_rl_repo")

from contextlib import ExitStack

import numpy as np

import concourse.bacc as bacc
import concourse.bass as bass
import concourse.mybir as mybir
from concourse import bass_utils
from concourse._compat import get_trn_type
from concourse.library_config import mlp

N_CORES = 8
NUM_BLOCKS, NUM_KV_HEADS, HEAD_DIM, BLOCK_SIZE = 4096, 8, 128, 16
BATCH, MAX_BLOCKS_PER_SEQ = 32, 128

ROW = NUM_KV_HEADS * BLOCK_SIZE * HEAD_DIM  # 16384 fp16 = one K or V half-block
N_ROWS = NUM_BLOCKS * 2                     # 8192
E_PER_CORE = (BATCH // N_CORES) * MAX_BLOCKS_PER_SEQ  # 512 entries per core
N_CHUNK = E_PER_CORE // 128                 # 4 gather chunks per K/V half
N_BUF = 3                                   # SBUF pipeline depth

_NC_CACHE = None


def build_nc() -> bass.Bass:
    # Bacc (not raw Bass): dma_gather's register operand needs the bacc
    # reg-alloc/lowering pass or walrus rejects the ISA encoding.
    nc = bacc.Bacc(get_trn_type() or "TRN2")

    kv = nc.dram_tensor("kv", [N_ROWS, ROW], mybir.dt.float16, kind="ExternalInput")
    bt = nc.dram_tensor("bt", [E_PER_CORE], mybir.dt.int32, kind="ExternalInput")
    scales = nc.dram_tensor("scales", [128, 2], mybir.dt.float32, kind="ExternalInput")
    # out block 0 stays zero (buffers are pre-zeroed); host keeps core 0's.
    out = nc.dram_tensor(
        "out", [E_PER_CORE + 1, 2, ROW], mybir.dt.float16, kind="ExternalOutput"
    )

    # bt viewed wrapped-16 (idx buffer layout) and partition-major-128 (scales)
    bt_w16 = bt.rearrange("(s p) -> p s", p=16)     # [16, 32]: bt[s*16+p]
    bt_p128 = bt.rearrange("(n p) -> p n", p=128)   # [128, 4]: bt[n*128+p]

    with (
        ExitStack() as stack,
        nc.Block() as block,
    ):
        bufs = [
            stack.enter_context(
                nc.sbuf_tensor(f"buf{i}", [128, 1, ROW], mybir.dt.float16)
            )
            for i in range(N_BUF)
        ]
        bt32 = stack.enter_context(nc.sbuf_tensor("bt32", [128, 32], mybir.dt.int32))
        btp32 = stack.enter_context(nc.sbuf_tensor("btp32", [128, 4], mybir.dt.int32))
        btf = stack.enter_context(nc.sbuf_tensor("btf", [128, 32], mybir.dt.float32))
        btpf = stack.enter_context(nc.sbuf_tensor("btpf", [128, 4], mybir.dt.float32))
        valid = stack.enter_context(nc.sbuf_tensor("valid", [128, 4], mybir.dt.float32))
        k16 = stack.enter_context(nc.sbuf_tensor("k16", [128, 32], mybir.dt.int16))
        v16 = stack.enter_context(nc.sbuf_tensor("v16", [128, 32], mybir.dt.int16))
        ksv = stack.enter_context(nc.sbuf_tensor("ksv", [128, 4], mybir.dt.float32))
        vsv = stack.enter_context(nc.sbuf_tensor("vsv", [128, 4], mybir.dt.float32))
        scl = stack.enter_context(nc.sbuf_tensor("scl", [128, 2], mybir.dt.float32))

        load_sem = stack.enter_context(nc.semaphore("load"))
        vchain = stack.enter_context(nc.semaphore("vchain"))
        scale_sem = stack.enter_context(nc.semaphore("scale"))
        # Per-buffer DMA sems: concurrent DMAs on one shared sem would make
        # intermediate values ambiguous (increments from different DMAs mix).
        gather_sems = [
            stack.enter_context(nc.semaphore(f"gather{i}")) for i in range(N_BUF)
        ]
        store_sems = [
            stack.enter_context(nc.semaphore(f"store{i}")) for i in range(N_BUF)
        ]

        def chunk_aps(c):
            half, n = divmod(c, N_CHUNK)  # 0..3 -> K, 4..7 -> V
            idx = (k16 if half == 0 else v16)[:, 8 * n : 8 * n + 8]
            sc = (ksv if half == 0 else vsv)[:, n : n + 1]
            dst = out[1 + 128 * n : 1 + 128 * (n + 1), half, :]
            return idx, sc, dst

        @block.sync
        def _(sync: bass.BassEngine):
            # Prolog loads: bt wrapped-16 replicated into all 8 partition
            # groups, bt partition-major, and the scale pair.
            with nc.allow_non_contiguous_dma(reason="2KB one-time index loads"):
                for g in range(8):
                    sync.dma_start(
                        bt32[16 * g : 16 * g + 16, :], bt_w16[:, :]
                    ).then_inc(load_sem, 16)
                sync.dma_start(btp32[:, :], bt_p128[:, :]).then_inc(load_sem, 16)
            sync.dma_start(scl[:, :], scales[:, :]).then_inc(load_sem, 16)
            # Store loop
            for c in range(2 * N_CHUNK):
                _, _, dst = chunk_aps(c)
                sync.wait_ge(scale_sem, c + 1)
                sync.dma_start(dst, bufs[c % N_BUF][:, 0, :]).then_inc(
                    store_sems[c % N_BUF], 16
                )
            for b in range(N_BUF):
                rounds = len([c for c in range(2 * N_CHUNK) if c % N_BUF == b])
                sync.wait_ge(store_sems[b], 16 * rounds)

        @block.vector
        def _(vector: bass.BassVectorEngine):
            vector.wait_ge(load_sem, 16 * 10)
            # Row indices: k = 2*bt, v = 2*bt + 1 (as int16, clamped >= 0).
            # Same-engine RAW chains need explicit sync (deep pipeline).
            vector.tensor_copy(btf[:, :], bt32[:, :]).then_inc(vchain, 1)
            vector.wait_ge(vchain, 1)
            vector.tensor_scalar_max(btf[:, :], btf[:, :], 0.0).then_inc(vchain, 1)
            vector.wait_ge(vchain, 2)
            vector.tensor_scalar_mul(k16[:, :], btf[:, :], 2.0).then_inc(vchain, 1)
            vector.tensor_scalar(
                v16[:, :], btf[:, :], 2.0, 1.0,
                op0=mybir.AluOpType.mult, op1=mybir.AluOpType.add,
            ).then_inc(vchain, 1)
            # Per-entry scales: (bt > 0) * {k,v}_scale, partition-major
            vector.tensor_copy(btpf[:, :], btp32[:, :]).then_inc(vchain, 1)
            vector.wait_ge(vchain, 5)
            vector.tensor_scalar(
                valid[:, :], btpf[:, :], 0.0, None, op0=mybir.AluOpType.is_gt
            ).then_inc(vchain, 1)
            vector.wait_ge(vchain, 6)
            vector.tensor_scalar_mul(ksv[:, :], valid[:, :], scl[:, 0:1]).then_inc(
                vchain, 1
            )
            vector.tensor_scalar_mul(vsv[:, :], valid[:, :], scl[:, 1:2]).then_inc(
                vchain, 1
            )
            vector.wait_ge(vchain, 8)
            # Dequant loop
            for c in range(2 * N_CHUNK):
                _, sc, _ = chunk_aps(c)
                buf = bufs[c % N_BUF]
                vector.wait_ge(gather_sems[c % N_BUF], 16 * (c // N_BUF + 1))
                vector.tensor_scalar_mul(buf[:, :, :], buf[:, :, :], sc).then_inc(
                    scale_sem, 1
                )

        @block.gpsimd
        def _(gpsimd: bass.BassGpSimd):
            gpsimd.load_library(mlp)
            gpsimd.wait_ge(vchain, 4)  # k16/v16 written
            for c in range(2 * N_CHUNK):
                idx, _, _ = chunk_aps(c)
                if c >= N_BUF:
                    gpsimd.wait_ge(store_sems[c % N_BUF], 16 * (c // N_BUF))
                gpsimd.dma_gather(
                    bufs[c % N_BUF][:, :, :], kv[:, :], idx, 128, 128, ROW
                ).then_inc(gather_sems[c % N_BUF], 16)

    nc.compile()
    return nc


def _get_nc() -> bass.Bass:
    global _NC_CACHE
    if _NC_CACHE is None:
        _NC_CACHE = build_nc()
    return _NC_CACHE


def _make_in_maps(inputs):
    kv = np.ascontiguousarray(np.asarray(inputs["kv_cache"])).view(np.float16)
    bt = np.asarray(inputs["block_tables"])
    k_scale = np.float32(inputs["k_scale"])
    v_scale = np.float32(inputs["v_scale"])

    kv_flat = kv.reshape(N_ROWS, ROW)
    scales = np.empty((128, 2), np.float32)
    scales[:, 0] = k_scale
    scales[:, 1] = v_scale

    seq_per_core = BATCH // N_CORES
    in_maps = []
    for c in range(N_CORES):
        bt_shard = np.ascontiguousarray(
            bt[seq_per_core * c : seq_per_core * (c + 1)]
            .reshape(-1)
            .astype(np.int32)
        )
        in_maps.append({"kv": kv_flat, "bt": bt_shard, "scales": scales})
    return in_maps


def _run(inputs, **kwargs) -> tuple[np.ndarray, "bass_utils.BassKernelResults"]:
    res = bass_utils.run_bass_kernel_spmd(
        _get_nc(), _make_in_maps(inputs), core_ids=list(range(N_CORES)), **kwargs
    )
    outs = [r["out"] for r in res.results]  # each [513, 2, ROW] fp16
    full = np.empty((BATCH * MAX_BLOCKS_PER_SEQ + 1, 2, ROW), np.float16)
    full[0] = outs[0][0]
    for c in range(N_CORES):
        full[1 + E_PER_CORE * c : 1 + E_PER_CORE * (c + 1)] = outs[c][1:]
    return (
        full.reshape(-1, 2, NUM_KV_HEADS, BLOCK_SIZE, HEAD_DIM),
        res,
    )


def kernel(**inputs) -> np.ndarray:
    out, _ = _run(inputs)
    return out


# revision 17
# speedup vs baseline: 425406.2529x; 425406.2529x over previous
"""Paged KV-cache gather + dequant kernel for 8 Trainium2 NeuronCores.

Problem: out[0] = zeros; out[1+i] = kv_cache[block_tables.flat[i]] * scale
(k_scale for the K half, v_scale for the V half), zeroed where the table
entry is <= 0.  Shapes: kv_cache [4096, 2, 8, 16, 128] fp16,
block_tables [32, 128] int, out [4097, 2, 8, 16, 128] fp16.

Sharding: batch across the 8 cores (4 sequences = 512 entries per core);
kv_cache replicated.  Per core the kernel views kv_cache as 8192 rows of
16384 fp16 (one row = one K or V half-block, 32 KB) and:
  1. loads block_tables, builds int16 row indices (2*bt for K, 2*bt+1 for V)
     wrapped in 16 partitions and replicated across the 8 GPSIMD cores,
  2. dma_gather's 128 rows per call into [128, 16384] SBUF tiles,
  3. multiplies by a per-partition scalar (valid * k/v_scale) on DVE,
  4. stores rows to the output shard with a strided HWDGE DMA.
Invalid entries gather row 0/1 and are zeroed by the scale; output block 0
is never written (ExternalOutput buffers are zero-initialized).
"""

import sys

if "/opt/trn_rl_repo" not in sys.path:
    sys.path.insert(0, "/opt/trn_rl_repo")

from contextlib import ExitStack

import numpy as np

import concourse.bacc as bacc
import concourse.bass as bass
import concourse.mybir as mybir
from concourse import bass_utils
from concourse._compat import get_trn_type
from concourse.library_config import mlp

N_CORES = 8
NUM_BLOCKS, NUM_KV_HEADS, HEAD_DIM, BLOCK_SIZE = 4096, 8, 128, 16
BATCH, MAX_BLOCKS_PER_SEQ = 32, 128

ROW = NUM_KV_HEADS * BLOCK_SIZE * HEAD_DIM  # 16384 fp16 = one K or V half-block
N_ROWS = NUM_BLOCKS * 2                     # 8192
E_PER_CORE = (BATCH // N_CORES) * MAX_BLOCKS_PER_SEQ  # 512 entries per core
N_CHUNK = E_PER_CORE // 128                 # 4 gather chunks per K/V half
N_BUF = 3                                   # SBUF pipeline depth

_NC_CACHE = None


def build_nc(n_reps: int = 1) -> bass.Bass:
    # Bacc (not raw Bass): dma_gather's register operand needs the bacc
    # reg-alloc/lowering pass or walrus rejects the ISA encoding.
    # n_reps > 1 unrolls the main loop for benchmarking (same output).
    nc = bacc.Bacc(get_trn_type() or "TRN2")

    kv = nc.dram_tensor("kv", [N_ROWS, ROW], mybir.dt.float16, kind="ExternalInput")
    bt = nc.dram_tensor("bt", [E_PER_CORE], mybir.dt.int32, kind="ExternalInput")
    scales = nc.dram_tensor("scales", [128, 2], mybir.dt.float32, kind="ExternalInput")
    # out block 0 stays zero (buffers are pre-zeroed); host keeps core 0's.
    out = nc.dram_tensor(
        "out", [E_PER_CORE + 1, 2, ROW], mybir.dt.float16, kind="ExternalOutput"
    )

    # bt viewed wrapped-16 (idx buffer layout) and partition-major-128 (scales)
    bt_w16 = bt.rearrange("(s p) -> p s", p=16)     # [16, 32]: bt[s*16+p]
    bt_p128 = bt.rearrange("(n p) -> p n", p=128)   # [128, 4]: bt[n*128+p]

    with (
        ExitStack() as stack,
        nc.Block() as block,
    ):
        bufs = [
            stack.enter_context(
                nc.sbuf_tensor(f"buf{i}", [128, 1, ROW], mybir.dt.float16)
            )
            for i in range(N_BUF)
        ]
        bt32 = stack.enter_context(nc.sbuf_tensor("bt32", [128, 32], mybir.dt.int32))
        btp32 = stack.enter_context(nc.sbuf_tensor("btp32", [128, 4], mybir.dt.int32))
        btf = stack.enter_context(nc.sbuf_tensor("btf", [128, 32], mybir.dt.float32))
        btpf = stack.enter_context(nc.sbuf_tensor("btpf", [128, 4], mybir.dt.float32))
        valid = stack.enter_context(nc.sbuf_tensor("valid", [128, 4], mybir.dt.float32))
        k16 = stack.enter_context(nc.sbuf_tensor("k16", [128, 32], mybir.dt.int16))
        v16 = stack.enter_context(nc.sbuf_tensor("v16", [128, 32], mybir.dt.int16))
        ksv = stack.enter_context(nc.sbuf_tensor("ksv", [128, 4], mybir.dt.float32))
        vsv = stack.enter_context(nc.sbuf_tensor("vsv", [128, 4], mybir.dt.float32))
        scl = stack.enter_context(nc.sbuf_tensor("scl", [128, 2], mybir.dt.float32))

        load_sem = stack.enter_context(nc.semaphore("load"))
        vchain = stack.enter_context(nc.semaphore("vchain"))
        scale_sem = stack.enter_context(nc.semaphore("scale"))
        # Per-buffer DMA sems: concurrent DMAs on one shared sem would make
        # intermediate values ambiguous (increments from different DMAs mix).
        gather_sems = [
            stack.enter_context(nc.semaphore(f"gather{i}")) for i in range(N_BUF)
        ]
        store_sems = [
            stack.enter_context(nc.semaphore(f"store{i}")) for i in range(N_BUF)
        ]

        def chunk_aps(g):
            half, n = divmod(g % (2 * N_CHUNK), N_CHUNK)  # 0..3 -> K, 4..7 -> V
            idx = (k16 if half == 0 else v16)[:, 8 * n : 8 * n + 8]
            sc = (ksv if half == 0 else vsv)[:, n : n + 1]
            dst = out[1 + 128 * n : 1 + 128 * (n + 1), half, :]
            return idx, sc, dst

        n_total = 2 * N_CHUNK * n_reps

        @block.sync
        def _(sync: bass.BassEngine):
            # Prolog loads: bt wrapped-16 replicated into all 8 partition
            # groups, bt partition-major, and the scale pair.
            with nc.allow_non_contiguous_dma(reason="2KB one-time index loads"):
                for g in range(8):
                    sync.dma_start(
                        bt32[16 * g : 16 * g + 16, :], bt_w16[:, :]
                    ).then_inc(load_sem, 16)
                sync.dma_start(btp32[:, :], bt_p128[:, :]).then_inc(load_sem, 16)
            sync.dma_start(scl[:, :], scales[:, :]).then_inc(load_sem, 16)
            # Store loop
            for c in range(n_total):
                _, _, dst = chunk_aps(c)
                sync.wait_ge(scale_sem, c + 1)
                sync.dma_start(dst, bufs[c % N_BUF][:, 0, :]).then_inc(
                    store_sems[c % N_BUF], 16
                )
            for b in range(N_BUF):
                rounds = len([c for c in range(n_total) if c % N_BUF == b])
                sync.wait_ge(store_sems[b], 16 * rounds)

        @block.vector
        def _(vector: bass.BassVectorEngine):
            vector.wait_ge(load_sem, 16 * 10)
            # Row indices: k = 2*bt, v = 2*bt + 1 (as int16, clamped >= 0).
            # Same-engine RAW chains need explicit sync (deep pipeline).
            vector.tensor_copy(btf[:, :], bt32[:, :]).then_inc(vchain, 1)
            vector.wait_ge(vchain, 1)
            vector.tensor_scalar_max(btf[:, :], btf[:, :], 0.0).then_inc(vchain, 1)
            vector.wait_ge(vchain, 2)
            vector.tensor_scalar_mul(k16[:, :], btf[:, :], 2.0).then_inc(vchain, 1)
            vector.tensor_scalar(
                v16[:, :], btf[:, :], 2.0, 1.0,
                op0=mybir.AluOpType.mult, op1=mybir.AluOpType.add,
            ).then_inc(vchain, 1)
            # Per-entry scales: (bt > 0) * {k,v}_scale, partition-major
            vector.tensor_copy(btpf[:, :], btp32[:, :]).then_inc(vchain, 1)
            vector.wait_ge(vchain, 5)
            vector.tensor_scalar(
                valid[:, :], btpf[:, :], 0.0, None, op0=mybir.AluOpType.is_gt
            ).then_inc(vchain, 1)
            vector.wait_ge(vchain, 6)
            vector.tensor_scalar_mul(ksv[:, :], valid[:, :], scl[:, 0:1]).then_inc(
                vchain, 1
            )
            vector.tensor_scalar_mul(vsv[:, :], valid[:, :], scl[:, 1:2]).then_inc(
                vchain, 1
            )
            vector.wait_ge(vchain, 8)
            # Dequant loop
            for c in range(n_total):
                _, sc, _ = chunk_aps(c)
                buf = bufs[c % N_BUF]
                vector.wait_ge(gather_sems[c % N_BUF], 16 * (c // N_BUF + 1))
                vector.tensor_scalar_mul(buf[:, :, :], buf[:, :, :], sc).then_inc(
                    scale_sem, 1
                )

        @block.gpsimd
        def _(gpsimd: bass.BassGpSimd):
            gpsimd.load_library(mlp)
            gpsimd.wait_ge(vchain, 4)  # k16/v16 written
            for c in range(n_total):
                idx, _, _ = chunk_aps(c)
                if c >= N_BUF:
                    gpsimd.wait_ge(store_sems[c % N_BUF], 16 * (c // N_BUF))
                gpsimd.dma_gather(
                    bufs[c % N_BUF][:, :, :], kv[:, :], idx, 128, 128, ROW
                ).then_inc(gather_sems[c % N_BUF], 16)

    nc.compile()
    return nc


def _get_nc() -> bass.Bass:
    global _NC_CACHE
    if _NC_CACHE is None:
        _NC_CACHE = build_nc()
    return _NC_CACHE


def _make_in_maps(inputs):
    kv = np.ascontiguousarray(np.asarray(inputs["kv_cache"])).view(np.float16)
    bt = np.asarray(inputs["block_tables"])
    k_scale = np.float32(inputs["k_scale"])
    v_scale = np.float32(inputs["v_scale"])

    kv_flat = kv.reshape(N_ROWS, ROW)
    scales = np.empty((128, 2), np.float32)
    scales[:, 0] = k_scale
    scales[:, 1] = v_scale

    seq_per_core = BATCH // N_CORES
    in_maps = []
    for c in range(N_CORES):
        bt_shard = np.ascontiguousarray(
            bt[seq_per_core * c : seq_per_core * (c + 1)]
            .reshape(-1)
            .astype(np.int32)
        )
        in_maps.append({"kv": kv_flat, "bt": bt_shard, "scales": scales})
    return in_maps


def _run(inputs, **kwargs) -> tuple[np.ndarray, "bass_utils.BassKernelResults"]:
    res = bass_utils.run_bass_kernel_spmd(
        _get_nc(), _make_in_maps(inputs), core_ids=list(range(N_CORES)), **kwargs
    )
    outs = [r["out"] for r in res.results]  # each [513, 2, ROW] fp16
    full = np.empty((BATCH * MAX_BLOCKS_PER_SEQ + 1, 2, ROW), np.float16)
    full[0] = outs[0][0]
    for c in range(N_CORES):
        full[1 + E_PER_CORE * c : 1 + E_PER_CORE * (c + 1)] = outs[c][1:]
    return (
        full.reshape(-1, 2, NUM_KV_HEADS, BLOCK_SIZE, HEAD_DIM),
        res,
    )


def kernel(**inputs) -> np.ndarray:
    out, _ = _run(inputs)
    return out
